# revision 10
# baseline (speedup 1.0000x reference)
"""Causal self-attention Trainium2 kernel (v2 - pipelined).

Sharding: 8 cores = (4 batches) x (2 head-groups of 8 heads).
Each core: projections for its 512 channels, causal attention for its 8
heads over its batch, partial out-projection over its 512 channels.
Host: sums the two partials per batch and adds the output bias.

Pipeline (v3):
  - input DMAs split per-chunk across the SP and ACT hardware DGE queues.
  - attention runs as TWO interleaved software pipelines (head-group
    hh=0 / hh=1 of the same oc-chunk); PE-queue order per round is
    [sc_A, sc_B, at_A, at_B, proj-unit], which hides the ACT exp latency
    and interleaves the remaining q/k/v projection work into the rounds
    so the exp window starts ~25us in.
  - causal masking via a PE matmul that accumulates NEG[j,c]=-1000*(c<j)
    into the 128-wide diagonal band of the scores psum (exp then yields
    exact zeros) - no vector-engine mask on the critical path.
  - causal narrowing: diagonal score tiles/attn*V accumulations compute
    only columns >= d*128; all-diagonal exp groups are narrowed too.
  - softmax normalization fully off the PE/ACT path: unnormalized head
    output is copied to attT, denominator rows staged on partition 64,
    DMA-relocated and reciprocal'd 4-rows-at-a-time on DVE, bounced
    through DRAM for a stride-0 partition-broadcast DMA, then one
    in-place [128,2048] bf16 multiply per oc-chunk.

Layouts on core (b = fixed batch, channels o in [0,512) local):
  xT   [128f, 8fc, 2048t] bf16 - loaded via DMA transpose from DRAM
  qT/kT [128o, 4oc, 2048t] bf16 - head h = oc*2+hh on partitions hh*64..+64 of chunk oc
  vx   [128t, 16tj, 8h*65] bf16 - v natural + ones column per head (softmax denom)
  scores^T tiles [128j, 512i] f32 psum -> exp on ACT (scale=1/8) -> bf16, masked
  attn^T accum psum [65, 512i] f32: rows 0..63 head out, row 64 denom
  out  psum [128t, 512c] f32 -> sbuf -> DRAM partial
"""

from contextlib import ExitStack

import ml_dtypes
import numpy as np

import concourse.bass as bass
import concourse.mybir as mybir
import concourse.tile as tile

P = 128
C = 1024  # d_model
CL = 512  # local channels (8 heads * 64)
D = 64  # head dim
NH = 8  # local heads
FC = C // P  # 8 f-chunks
OC = CL // P  # 4 o-chunks
F32 = mybir.dt.float32
BF16 = mybir.dt.bfloat16
AF = mybir.ActivationFunctionType
GROUP = 2  # score jt-tiles per exp call (2 psum banks, double buffered)


def _emit(nc, tc, ctx, T):
    NT = T // P  # 128-token chunks
    T4 = T // 512  # 512-token chunks

    xb = nc.dram_tensor("xb", [T, C], BF16, kind="ExternalInput")
    wq_d = nc.dram_tensor("wq", [C, CL], BF16, kind="ExternalInput")
    wk_d = nc.dram_tensor("wk", [C, CL], BF16, kind="ExternalInput")
    wv_d = nc.dram_tensor("wv", [C, CL], BF16, kind="ExternalInput")
    wo_d = nc.dram_tensor("wo", [CL, C], BF16, kind="ExternalInput")
    bq_d = nc.dram_tensor("bq", [CL], F32, kind="ExternalInput")
    bk_d = nc.dram_tensor("bk", [CL], F32, kind="ExternalInput")
    bv_d = nc.dram_tensor("bv", [CL], BF16, kind="ExternalInput")
    ident_d = nc.dram_tensor("ident", [P, P], BF16, kind="ExternalInput")
    neg_d = nc.dram_tensor("neg", [P, P], BF16, kind="ExternalInput")
    outp = nc.dram_tensor("outp", [T, C], F32, kind="ExternalOutput")
    den_dram = nc.dram_tensor("den_scr", [2, OC, 512], F32, kind="Internal")
    rec_dram = nc.dram_tensor("rec_scr", [NH, T4, 512], BF16, kind="Internal")

    const = ctx.enter_context(tc.tile_pool(name="const", bufs=1))
    ones1 = const.tile([1, P], BF16)
    nc.gpsimd.memset(ones1[:], 1.0)

    # small constants ride the SP queue first (tiny, ~5KB total)
    bq_sb = const.tile([P, OC], F32)
    nc.sync.dma_start(bq_sb[:], bq_d.rearrange("(oc p) -> p oc", p=P))
    bk_sb = const.tile([P, OC], F32)
    nc.sync.dma_start(bk_sb[:], bk_d.rearrange("(oc p) -> p oc", p=P))
    bv_sb = const.tile([1, CL], BF16)
    nc.sync.dma_start(bv_sb[:], bv_d.rearrange("(a c) -> a c", a=1))
    ident_sb = const.tile([P, P], BF16)
    nc.sync.dma_start(ident_sb[:], ident_d[:])
    neg_sb = const.tile([P, P], BF16)
    nc.sync.dma_start(neg_sb[:], neg_d[:])

    qkv = ctx.enter_context(tc.tile_pool(name="qkv", bufs=1))
    qT = qkv.tile([P, OC, T], BF16)
    kT = qkv.tile([P, OC, T], BF16)
    vx = qkv.tile([P, NT, NH * 65], BF16)
    vx5 = vx.rearrange("p n (h u) -> p n h u", u=65)
    nc.gpsimd.memset(vx5[:, :, :, 64:65], 1.0)

    # ---------------- projections (as interleavable units) ----------------
    wpool = ctx.enter_context(tc.tile_pool(name="wpool", bufs=1))
    xT_pool = ctx.enter_context(tc.tile_pool(name="xT_pool", bufs=1))
    pj_ps = ctx.enter_context(tc.tile_pool(name="pj_ps", bufs=2, space="PSUM"))

    wq_sb = wpool.tile([P, FC, CL], BF16)
    wk_sb = wpool.tile([P, FC, CL], BF16)
    wv_sb = wpool.tile([P, FC, CL], BF16)
    xT = xT_pool.tile([P, FC, T], BF16)
    xbr = xb.rearrange("t (fc p) -> t fc p", p=P)
    wq_r = wq_d.rearrange("(fc p) o -> p fc o", p=P)
    wk_r = wk_d.rearrange("(fc p) o -> p fc o", p=P)
    wv_r = wv_d.rearrange("(fc p) o -> p fc o", p=P)
    # 3-way queue split so the transposed x lands as early as possible:
    # SP: wq then xT chunks 0-2; ACT: xT chunks 3-7; GpSimd software DGE
    # carries wk/wv in parallel.
    for fc in range(FC):
        nc.sync.dma_start(wq_sb[:, fc, :], wq_r[:, fc, :])
    for fc in range(3):
        nc.sync.dma_start(xT[:, fc, :], xbr[:, fc, :], transpose=True)
    for fc in range(3, FC):
        nc.scalar.dma_start(xT[:, fc, :], xbr[:, fc, :], transpose=True)
    for fc in range(FC):
        nc.gpsimd.dma_start(wk_sb[:, fc, :], wk_r[:, fc, :])
    for fc in range(FC):
        nc.gpsimd.dma_start(wv_sb[:, fc, :], wv_r[:, fc, :])

    def qk_unit(oc, which, tt):
        # one [128,512] token-chunk of the q or k projection for chunk oc
        w_sb, b_sb, dT = (
            (wq_sb, bq_sb, qT) if which == 0 else (wk_sb, bk_sb, kT)
        )
        ps = pj_ps.tile([P, 512], F32, tag="pj", name=f"pj{which}_{oc}_{tt}")
        for fc in range(FC):
            nc.tensor.matmul(
                ps[:],
                w_sb[:, fc, oc * P : (oc + 1) * P],
                xT[:, fc, tt * 512 : (tt + 1) * 512],
                start=(fc == 0),
                stop=(fc == FC - 1),
            )
        nc.vector.tensor_scalar_add(
            dT[:, oc, tt * 512 : (tt + 1) * 512], ps[:], b_sb[:, oc : oc + 1]
        )

    def v_unit(s):
        ps = pj_ps.tile([P, 512], F32, tag="pj", name=f"pjv{s}")
        for fc in range(FC):
            nc.tensor.matmul(
                ps[:],
                xT[:, fc, s * P : (s + 1) * P],
                wv_sb[:, fc, :],
                start=(fc == 0),
                stop=False,
            )
        nc.tensor.matmul(ps[:], ones1[:], bv_sb[:], start=False, stop=True)
        nc.vector.tensor_copy(
            vx5[:, s, :, 0:64],
            ps[:].rearrange("p (h d) -> p h d", d=D),
        )

    # work queue: remaining proj units, pulled one per attention round.
    proj_units = []
    for s in range(4, NT):
        proj_units.append(lambda s=s: v_unit(s))
    for oc in range(1, OC):
        for which in range(2):
            for tt in range(T4):
                proj_units.append(
                    lambda oc=oc, which=which, tt=tt: qk_unit(oc, which, tt)
                )

    # upfront: q/k for oc0 and the first v tiles, so round 0 can start.
    for which in range(2):
        for tt in range(T4):
            qk_unit(0, which, tt)
    for s in range(4):
        v_unit(s)

    # ---------------- attention ----------------
    wo_pool = ctx.enter_context(tc.tile_pool(name="wo_pool", bufs=1))
    attT_pool = ctx.enter_context(tc.tile_pool(name="attT_pool", bufs=1))
    wo_sb = wo_pool.tile([P, OC, C], BF16)
    nc.sync.dma_start(wo_sb[:], wo_d.rearrange("(oc p) c -> p oc c", p=P))
    attT = attT_pool.tile([P, OC, T], BF16)

    # global round list: for each oc, the 15 groups of its 4 ic-chunks;
    # stream A = hh 0, stream B = hh 1 run the same schedule.
    rounds = []
    for oc in range(OC):
        for ic in range(T4):
            njt = ic * 4 + 4
            jts = list(range(njt))
            gs = [jts[g : g + GROUP] for g in range(0, njt, GROUP)]
            for gi, grp in enumerate(gs):
                rounds.append(
                    dict(oc=oc, ic=ic, njt=njt, grp=grp, last=(gi == len(gs) - 1))
                )
    NR = len(rounds)

    recB_pool = ctx.enter_context(tc.tile_pool(name="recB_pool", bufs=1))
    recB = recB_pool.tile([P, OC, T], BF16)

    with (
        tc.tile_pool(name="exp_pool", bufs=4) as exp_pool,
        tc.tile_pool(name="den_pool", bufs=2) as den_pool,
        tc.tile_pool(name="sc_ps", bufs=2, space="PSUM") as sc_ps_pool,
        tc.tile_pool(name="at_ps", bufs=2, space="PSUM") as at_ps_pool,
    ):
        ex_live = {}  # (hh, r) -> exp tile
        at_live = {}  # hh -> current accumulation tile

        def emit_sc(hh, r):
            st = rounds[r]
            oc, ic = st["oc"], st["ic"]
            base = hh * 64
            n = len(st["grp"])
            sc = sc_ps_pool.tile([P, GROUP, 512], F32, tag="sc", name=f"sc{hh}_{r}")
            ex = exp_pool.tile([P, GROUP, 512], BF16, tag="ex", name=f"ex{hh}_{r}")
            for si, jt in enumerate(st["grp"]):
                d = jt - ic * 4
                lo = d * P if d > 0 else 0
                diag = d >= 0
                nc.tensor.matmul(
                    sc[:, si, lo:512],
                    kT[base : base + D, oc, jt * P : (jt + 1) * P],
                    qT[base : base + D, oc, ic * 512 + lo : (ic + 1) * 512],
                    start=True,
                    stop=not diag,
                )
                if diag:
                    # accumulate -1000 into the invalid (c < j) half of the
                    # 128-wide diagonal band; exp then yields exact zeros
                    nc.tensor.matmul(
                        sc[:, si, lo : lo + P],
                        ident_sb[:],
                        neg_sb[:],
                        start=False,
                        stop=True,
                    )
            dmin = min(jt - ic * 4 for jt in st["grp"])
            elo = dmin * P if dmin >= 2 else 0
            nc.scalar.activation(
                ex[:, 0:n, elo:512], sc[:, 0:n, elo:512], AF.Exp, scale=0.125
            )
            ex_live[(hh, r)] = ex

        def emit_at(hh, r):
            st = rounds[r]
            oc, ic, njt = st["oc"], st["ic"], st["njt"]
            h = oc * 2 + hh
            ex = ex_live.pop((hh, r))
            if st["grp"][0] == 0:
                at_live[hh] = at_ps_pool.tile([P, 512], F32, tag="at", name=f"at{hh}_{r}")
            at = at_live[hh]
            for si, jt in enumerate(st["grp"]):
                d = jt - ic * 4
                lo = d * P if d > 0 else 0
                nc.tensor.matmul(
                    at[0:65, lo:512],
                    vx5[:, jt, h, :],
                    ex[:, si, lo:512],
                    start=(jt == 0),
                    stop=(jt == njt - 1),
                )
            return at

        def emit_iter_end(hh, r, at):
            # evacuate the unnormalized head output and stage the softmax
            # denominator row on partition 64 (DVE is lane-aligned); the
            # reciprocal chain is deferred two rounds so this DVE burst
            # never delays the at-bank recycling of the current rounds.
            st = rounds[r]
            oc, ic = st["oc"], st["ic"]
            base = hh * 64
            nc.vector.tensor_copy(
                attT[base : base + D, oc, ic * 512 : (ic + 1) * 512],
                at[0:64, :],
            )
            dstg = den_live.get(hh)
            if dstg is None:
                dstg = den_pool.tile(
                    [P, T4, 512], F32, tag=f"dstg{hh}", name=f"dstg{hh}"
                )
                den_live[hh] = dstg
            nc.vector.tensor_copy(dstg[64:65, ic, :], at[64:65, :])
            if ic == T4 - 1:
                den_live[hh] = None
                return dstg
            return None

        def emit_norm(hh, oc, dstg):
            # batched reciprocal of the four denominator rows, bounced
            # through DRAM for the partition-broadcast back into recB.
            # All scratch DMAs ride the in-order SP queue.
            h = oc * 2 + hh
            base = hh * 64
            den = den_pool.tile([T4, 512], F32, tag=f"den{hh}", name=f"den{hh}")
            nc.sync.dma_start(den[:], dstg[64:65, :, :])
            rec = den_pool.tile([T4, 512], F32, tag=f"rec{hh}", name=f"rec{hh}")
            nc.vector.reciprocal(rec[:], den[:])
            recb = den_pool.tile([T4, 512], BF16, tag=f"recb{hh}", name=f"recb{hh}")
            nc.vector.tensor_copy(recb[:], rec[:])
            nc.sync.dma_start(rec_dram[h], recb[:])
            nc.sync.dma_start(
                recB[base : base + D, oc, :].rearrange("p (i f) -> p i f", f=512),
                rec_dram[h].unsqueeze(0).to_broadcast([D, T4, 512]),
            )
            if hh == 1:
                # both head-streams of this oc done: normalize in place
                nc.vector.tensor_mul(attT[:, oc, :], attT[:, oc, :], recB[:, oc, :])

        den_live = {}
        pend = []
        emit_sc(0, 0)
        emit_sc(1, 0)
        for r in range(1, NR + 1):
            if r < NR:
                emit_sc(0, r)
                emit_sc(1, r)
            atA = emit_at(0, r - 1)
            atB = emit_at(1, r - 1)
            if proj_units:
                proj_units.pop(0)()
            if rounds[r - 1]["last"]:
                dA = emit_iter_end(0, r - 1, atA)
                dB = emit_iter_end(1, r - 1, atB)
                if dA is not None:
                    oc = rounds[r - 1]["oc"]
                    pend.append([r + 2, 0, oc, dA])
                    pend.append([r + 2, 1, oc, dB])
            while pend and (pend[0][0] <= r or r == NR):
                _, hh_, oc_, d_ = pend.pop(0)
                emit_norm(hh_, oc_, d_)

    # ---------------- out-projection ----------------
    with (
        tc.tile_pool(name="op_ps", bufs=4, space="PSUM") as op_ps,
        tc.tile_pool(name="ob_pool", bufs=6) as ob_pool,
    ):
        for s16 in range(NT):
            for ch in range(2):
                ps = op_ps.tile([P, 512], F32)
                for oc in range(OC):
                    nc.tensor.matmul(
                        ps[:],
                        attT[:, oc, s16 * P : (s16 + 1) * P],
                        wo_sb[:, oc, ch * 512 : (ch + 1) * 512],
                        start=(oc == 0),
                        stop=(oc == OC - 1),
                    )
                ob = ob_pool.tile([P, 512], F32)
                if (s16 + ch) % 2 == 0:
                    nc.scalar.copy(ob[:], ps[:])
                else:
                    nc.vector.tensor_copy(ob[:], ps[:])
                eng = nc.sync if (s16 + ch) % 2 == 0 else nc.scalar
                eng.dma_start(
                    outp[s16 * P : (s16 + 1) * P, ch * 512 : (ch + 1) * 512],
                    ob[:],
                )


def build(T=2048):
    nc = bass.Bass()
    with tile.TileContext(nc) as tc:
        with ExitStack() as ctx:
            _emit(nc, tc, ctx, T)
    return nc


def make_ident_neg():
    j = np.arange(P)[:, None]
    u = np.arange(P)[None, :]
    ident = np.eye(P).astype(ml_dtypes.bfloat16)
    neg = ((u < j) * -1000.0).astype(ml_dtypes.bfloat16)
    return ident, neg


def make_in_maps(x, wq, bq, wk, bk, wv, bv, wo):
    bf = ml_dtypes.bfloat16
    ident, neg = make_ident_neg()
    in_maps = []
    for c in range(8):
        b, g = c // 2, c % 2
        sl = slice(g * CL, (g + 1) * CL)
        in_maps.append(
            {
                "xb": np.ascontiguousarray(x[b]).astype(bf),
                "wq": np.ascontiguousarray(wq[:, sl]).astype(bf),
                "wk": np.ascontiguousarray(wk[:, sl]).astype(bf),
                "wv": np.ascontiguousarray(wv[:, sl]).astype(bf),
                "wo": np.ascontiguousarray(wo[sl, :]).astype(bf),
                "bq": np.ascontiguousarray(bq[sl]),
                "bk": np.ascontiguousarray(bk[sl]),
                "bv": np.ascontiguousarray(bv[sl]).astype(bf),
                "ident": ident,
                "neg": neg,
            }
        )
    return in_maps


_cache = {}


def _split_multi_waits(bir_json: bytes) -> bytes:
    """Split instructions carrying >1 sync waits into single-wait NoOp
    chains on the same engine queue.  The TPB instruction encoding has one
    wait slot; this walrus build refuses multi-wait instructions instead
    of splitting them itself."""
    import orjson

    m = orjson.loads(bir_json)
    n = 0
    for fn in m.get("functions", []):
        for blk in fn.get("blocks", []):
            out = []
            for inst in blk.get("instructions", []):
                si = inst.get("sync_info")
                waits = si.get("on_wait") if si else None
                if waits and len(waits) > 1:
                    for w in waits[:-1]:
                        n += 1
                        out.append(
                            {
                                "debug": inst.get("debug", {}),
                                "engine": inst["engine"],
                                "ins": [],
                                "outs": [],
                                "name": f"{inst['name']}_sw{n}",
                                "opcode": "NoOp",
                                "text_hint": "split_wait",
                                "sync_info": {"on_wait": [w], "on_update": []},
                            }
                        )
                    si["on_wait"] = [waits[-1]]
                out.append(inst)
            blk["instructions"] = out
    return orjson.dumps(m)


def _install_compile_patch():
    import concourse.bass_utils as bu

    if getattr(bu, "_split_waits_patched", False):
        return
    orig = bu.compile_bir_kernel

    def patched(bir_json, tmpdir, neff_name="file.neff"):
        return orig(_split_multi_waits(bir_json), tmpdir, neff_name)

    bu.compile_bir_kernel = patched
    bu._split_waits_patched = True
    try:
        import concourse.bass2jax as b2j

        b2j.compile_bir_kernel = patched
    except ImportError:
        pass


def kernel(x, wq, bq, wk, bk, wv, bv, wo, bo):
    from concourse.bass_utils import run_bass_kernel_spmd

    _install_compile_patch()

    x = np.asarray(x, np.float32)
    args = [np.asarray(a, np.float32) for a in (wq, bq, wk, bk, wv, bv, wo, bo)]
    wq, bq, wk, bk, wv, bv, wo, bo = args
    B, T, _ = x.shape

    if "nc" not in _cache:
        _cache["nc"] = build(T)
    nc = _cache["nc"]

    in_maps = make_in_maps(x, wq, bq, wk, bk, wv, bv, wo)
    res = run_bass_kernel_spmd(nc, in_maps, core_ids=list(range(8)))
    out = np.empty((B, T, C), np.float32)
    for b in range(B):
        out[b] = res.results[2 * b]["outp"] + res.results[2 * b + 1]["outp"] + bo
    return out
